# revision 5
# baseline (speedup 1.0000x reference)
"""Trainium2 Bass kernel for the masked-LSTM MemoryAgent problem.

Contract: kernel(**inputs) takes the FULL unsharded inputs (as produced by
setup_inputs()) and returns the full outputs (out [T*B, A+1], hT [B, H],
cT [B, H]) exactly like the reference.

Strategy (8 NeuronCores, data-parallel over the batch/env axis B):
  - Each core owns BL = B/8 = 32 envs and runs the full T=512 sequential scan.
  - Everything on-chip uses a "transposed" layout: partitions = gate/hidden
    units, free dim = (time_local, env).  All elementwise tiles are [128, 32].
  - The input projection pre = x @ W_ih.T (+ biases + done-mask offsets) is
    produced on the tensor engine in batches of 8 timesteps directly into the
    PSUM tiles that the recurrent matmuls later accumulate into, so `pre`
    never round-trips through HBM (memory-optimal: x is read once).
  - The done-mask is folded in algebraically:
      * f-gate:  sigma(gates_f - 1e9*done_t)      == sigma(gates_f) * m_t
      * h-mask:  h_in(t+1) = sigma(gates_o - 1e9*done_{t+1}) * tanh(c)
    so no mask tensors or extra elementwise ops are needed in the scan.
  - The policy/value heads are folded into two packed matmuls (one stationary
    [128,128] for both first layers, one [128,9] block-diagonal for both
    second layers) batched over 8 timesteps.
"""

import os
import sys
import threading

import numpy as np

sys.path.insert(0, "/opt/trn_rl_repo")

import concourse.bass as bass  # noqa: E402
import concourse.tile as tile  # noqa: E402
from concourse import bacc, mybir  # noqa: E402
from contextlib import ExitStack  # noqa: E402

T, B, F, H, A, MH = 512, 256, 512, 128, 8, 64
G4 = 4 * H  # 512
N_CORES = 8
BL = B // N_CORES  # 32 envs per core
DT = mybir.dt.float32
NEG = -1.0e9

# gate row ranges in W_ih / W_hh (PyTorch order i, f, g, o)
GI, GF, GG, GO = 0, 1, 2, 3
GATE_ROWS = {GI: (0, 128), GF: (128, 256), GG: (256, 384), GO: (384, 512)}

TSTEPS_PER_TILE = 8
COLS = TSTEPS_PER_TILE * BL  # 256 free columns per psum tile half


def build_program(nt_steps=T):
    """Emit the per-core SPMD program. Returns the compiled bacc module."""
    n_tiles = nt_steps // TSTEPS_PER_TILE
    NTB = nt_steps * BL

    nc = bacc.Bacc(
        "TRN2", target_bir_lowering=False, debug=False, num_devices=N_CORES
    )

    # ---- DRAM I/O ----
    xT_d = nc.dram_tensor("xT", [F, NTB], DT, kind="ExternalInput").ap()
    aug_d = nc.dram_tensor("aug", [3, NTB], DT, kind="ExternalInput").ap()
    wih_d = nc.dram_tensor("wih", [128, 16 * 128], DT, kind="ExternalInput").ap()
    whh_d = nc.dram_tensor("whh", [128, 512], DT, kind="ExternalInput").ap()
    augw_d = nc.dram_tensor("augw", [3, 5 * 128], DT, kind="ExternalInput").ap()
    w1_d = nc.dram_tensor("w1", [128, 128], DT, kind="ExternalInput").ap()
    b1_d = nc.dram_tensor("b1", [128, 1], DT, kind="ExternalInput").ap()
    w2_d = nc.dram_tensor("w2", [128, A + 1], DT, kind="ExternalInput").ap()
    b2_d = nc.dram_tensor("b2", [A + 1, 1], DT, kind="ExternalInput").ap()
    h0_d = nc.dram_tensor("h0m", [128, BL], DT, kind="ExternalInput").ap()
    c0_d = nc.dram_tensor("c0", [128, BL], DT, kind="ExternalInput").ap()

    out9_d = nc.dram_tensor("out9", [A + 1, NTB], DT, kind="ExternalOutput").ap()
    hT_d = nc.dram_tensor("hT", [128, BL], DT, kind="ExternalOutput").ap()
    cT_d = nc.dram_tensor("cT", [128, BL], DT, kind="ExternalOutput").ap()

    SIG = mybir.ActivationFunctionType.Sigmoid
    TANH = mybir.ActivationFunctionType.Tanh
    MUL = mybir.AluOpType.mult
    ADD = mybir.AluOpType.add

    with tile.TileContext(nc) as tc, ExitStack() as ctx:
        consts = ctx.enter_context(tc.tile_pool(name="consts", bufs=1))
        xpool = ctx.enter_context(tc.tile_pool(name="xsb", bufs=8))
        augpool = ctx.enter_context(tc.tile_pool(name="augsb", bufs=2))
        hring_p = ctx.enter_context(tc.tile_pool(name="hring", bufs=2))
        state_p = ctx.enter_context(tc.tile_pool(name="state", bufs=3))
        tmp_p = ctx.enter_context(tc.tile_pool(name="tmp", bufs=3))
        s1_p = ctx.enter_context(tc.tile_pool(name="s1", bufs=2))
        s2_p = ctx.enter_context(tc.tile_pool(name="s2", bufs=2))
        pA_p = ctx.enter_context(tc.tile_pool(name="pA", bufs=2, space="PSUM"))
        pB_p = ctx.enter_context(tc.tile_pool(name="pB", bufs=2, space="PSUM"))
        pC_p = ctx.enter_context(tc.tile_pool(name="pC", bufs=2, space="PSUM"))
        pH1_p = ctx.enter_context(tc.tile_pool(name="pH1", bufs=1, space="PSUM"))
        pH2_p = ctx.enter_context(tc.tile_pool(name="pH2", bufs=1, space="PSUM"))

        # ---- load constants ----
        wih = consts.tile([128, 16 * 128], DT)
        whh = consts.tile([128, 512], DT)
        augw = consts.tile([3, 5 * 128], DT)
        w1 = consts.tile([128, 128], DT)
        b1 = consts.tile([128, 1], DT)
        w2 = consts.tile([128, A + 1], DT)
        b2 = consts.tile([A + 1, 1], DT)
        nc.sync.dma_start(wih[:], wih_d[:])
        nc.sync.dma_start(whh[:], whh_d[:])
        nc.sync.dma_start(augw[:], augw_d[:])
        nc.sync.dma_start(w1[:], w1_d[:])
        nc.sync.dma_start(b1[:], b1_d[:])
        nc.sync.dma_start(w2[:], w2_d[:])
        nc.sync.dma_start(b2[:], b2_d[:])

        h_init = state_p.tile([128, BL], DT, tag="h")
        c_init = state_p.tile([128, BL], DT, tag="c")
        nc.sync.dma_start(h_init[:], h0_d[:])
        nc.sync.dma_start(c_init[:], c0_d[:])

        # psum tiles per production tile j, created lazily
        psA = {}
        psB = {}
        psC = {}
        hrings = {}

        def dma_loads(j):
            """DMA x chunks + aug rows for tile j. Returns tiles."""
            jc = j * COLS
            xs = []
            for kc in range(4):
                t_ = xpool.tile([128, COLS], DT, tag=f"x{kc}")
                nc.sync.dma_start(
                    t_[:], xT_d[kc * 128 : (kc + 1) * 128, jc : jc + COLS]
                )
                xs.append(t_)
            ag = augpool.tile([3, COLS], DT, tag="aug")
            nc.sync.dma_start(ag[:], aug_d[:, jc : jc + COLS])
            return xs, ag

        def production_chunk(j, step_idx, xs, ag):
            """Emit a slice of tile-j pre-production, spread across 8 calls
            (step_idx 0..7)."""
            pa, pb, pc = psA[j], psB[j], psC[j]
            # region list: (psum_ap_256, gate_idx)
            regions = [
                (pa[:, 0:COLS], GI),
                (pa[:, COLS : 2 * COLS], GF),
                (pb[:, 0:COLS], GO),
                (pb[:, COLS : 2 * COLS], GG),
                (pc[:, 0:COLS], GO),  # o' tile
            ]
            # 5 regions x (4 x-chunks + 1 aug) = 25 matmuls; spread over
            # step_idx: do region r chunk kc when (r*5 + kc_slot) % 8 == ...
            # simpler: per step_idx emit a fixed sublist.
            all_mms = []
            for r, (ap_, g) in enumerate(regions):
                for kc in range(4):
                    all_mms.append(("x", r, ap_, g, kc))
                all_mms.append(("aug", r, ap_, g, None))
            # split into 8 roughly equal groups, preserving order
            n = len(all_mms)
            lo = (n * step_idx) // 8
            hi = (n * (step_idx + 1)) // 8
            for kind, r, ap_, g, kc in all_mms[lo:hi]:
                if kind == "x":
                    lhs = wih[:, (g * 4 + kc) * 128 : (g * 4 + kc + 1) * 128]
                    # start=True exactly once per psum tile (regions 0/2/4 are
                    # the first region of tiles A/B/C): it marks the whole
                    # 2KiB bank pending-zero, so only the first matmul into
                    # each bank may carry it.
                    nc.tensor.matmul(
                        ap_,
                        lhs,
                        xs[kc][:],
                        start=(kc == 0 and r in (0, 2, 4)),
                        stop=False,
                        skip_group_check=True,
                    )
                else:
                    # aug matmul: region r -> augw col block r
                    awr = 4 if r == 4 else (GI, GF, GO, GG)[r]
                    lhs = augw[:, awr * 128 : (awr + 1) * 128]
                    nc.tensor.matmul(
                        ap_,
                        lhs,
                        ag[:],
                        start=False,
                        stop=False,
                        skip_group_check=True,
                    )

        def heads(j):
            """Heads for completed hring tile j; returns list of emit-closures
            to spread across steps."""
            hr = hrings.pop(j)
            jc = j * COLS

            def part1():
                ph1 = pH1_p.tile([128, COLS], DT, tag="ph1")
                nc.tensor.matmul(ph1[:], w1[:], hr[:], start=True, stop=True)
                heads._ph1 = ph1

            def part2():
                s1 = s1_p.tile([128, COLS], DT, tag="s1")
                nc.scalar.activation(
                    s1[:], heads._ph1[:], TANH, bias=b1[:, 0:1]
                )
                heads._s1 = s1

            def part3():
                ph2 = pH2_p.tile([A + 1, COLS], DT, tag="ph2")
                nc.tensor.matmul(
                    ph2[:], w2[:], heads._s1[:], start=True, stop=True
                )
                heads._ph2 = ph2

            def part4():
                s2 = s2_p.tile([A + 1, COLS], DT, tag="s2")
                nc.vector.tensor_scalar_add(s2[:], heads._ph2[:], b2[:, 0:1])
                nc.sync.dma_start(out9_d[:, jc : jc + COLS], s2[:])

            return [part1, part2, part3, part4]

        # ---- prologue: tile 0 loads + production ----
        xs0, ag0 = dma_loads(0)
        psA[0] = pA_p.tile([128, 512], DT, tag="pA", name="psA")
        psB[0] = pB_p.tile([128, 512], DT, tag="pB", name="psB")
        psC[0] = pC_p.tile([128, COLS], DT, tag="pC", name="psC")
        hrings[0] = hring_p.tile([128, COLS], DT, tag="hr", name="hring")
        for s in range(8):
            production_chunk(0, s, xs0, ag0)

        h_prev, c_prev = h_init, c_init
        pending_heads = []
        pending_prod = None  # (j, xs, ag)

        for t in range(nt_steps):
            j, tl = t // TSTEPS_PER_TILE, t % TSTEPS_PER_TILE
            c0_ = tl * BL
            pa, pb, pc = psA[j], psB[j], psC[j]
            last = t == nt_steps - 1

            # --- interleave: start next tile's loads/psum at step 0 ---
            if tl == 0 and j + 1 < n_tiles:
                xs, ag = dma_loads(j + 1)
                psA[j + 1] = pA_p.tile([128, 512], DT, tag="pA", name="psA")
                psB[j + 1] = pB_p.tile([128, 512], DT, tag="pB", name="psB")
                psC[j + 1] = pC_p.tile([128, COLS], DT, tag="pC", name="psC")
                hrings[j + 1] = hring_p.tile([128, COLS], DT, tag="hr", name="hring")
                pending_prod = (j + 1, xs, ag)

            # --- recurrent matmuls (the latency-critical chain) ---
            nc.tensor.matmul(
                pa[:, c0_ : c0_ + BL], whh[:, 0:128], h_prev[:],
                start=False, stop=False, skip_group_check=True,
            )
            nc.tensor.matmul(
                pa[:, COLS + c0_ : COLS + c0_ + BL], whh[:, 128:256], h_prev[:],
                start=False, stop=False, skip_group_check=True,
            )
            nc.tensor.matmul(
                pb[:, COLS + c0_ : COLS + c0_ + BL], whh[:, 256:384], h_prev[:],
                start=False, stop=False, skip_group_check=True,
            )
            nc.tensor.matmul(
                pb[:, c0_ : c0_ + BL], whh[:, 384:512], h_prev[:],
                start=False, stop=False, skip_group_check=True,
            )
            if not last:
                nc.tensor.matmul(
                    pc[:, c0_ : c0_ + BL], whh[:, 384:512], h_prev[:],
                    start=False, stop=False, skip_group_check=True,
                )

            # --- activations ---
            sif = tmp_p.tile([128, 2 * BL], DT, tag="sif")
            pa_if = pa.rearrange("p (g n) -> p g n", g=2)[:, :, c0_ : c0_ + BL]
            nc.scalar.activation(sif[:].rearrange("p (g n) -> p g n", g=2), pa_if, SIG)
            g_sb = tmp_p.tile([128, BL], DT, tag="g")
            nc.scalar.activation(g_sb[:], pb[:, COLS + c0_ : COLS + c0_ + BL], TANH)
            o_sb = tmp_p.tile([128, BL], DT, tag="o")
            nc.scalar.activation(o_sb[:], pb[:, c0_ : c0_ + BL], SIG)
            if not last:
                op_sb = tmp_p.tile([128, BL], DT, tag="op")
                nc.scalar.activation(op_sb[:], pc[:, c0_ : c0_ + BL], SIG)

            # --- cell update (DVE) ---
            t1 = tmp_p.tile([128, BL], DT, tag="t1")
            nc.vector.tensor_tensor(t1[:], sif[:, BL : 2 * BL], c_prev[:], MUL)
            t2 = tmp_p.tile([128, BL], DT, tag="t2")
            nc.vector.tensor_tensor(t2[:], sif[:, 0:BL], g_sb[:], MUL)
            c_new = state_p.tile([128, BL], DT, tag="c")
            nc.vector.tensor_tensor(c_new[:], t1[:], t2[:], ADD)
            tc_sb = tmp_p.tile([128, BL], DT, tag="tc")
            nc.scalar.activation(tc_sb[:], c_new[:], TANH)
            if not last:
                h_new = state_p.tile([128, BL], DT, tag="h")
                nc.vector.tensor_tensor(h_new[:], op_sb[:], tc_sb[:], MUL)
            # unmasked h for outputs/heads
            nc.vector.tensor_tensor(
                hrings[j][:, c0_ : c0_ + BL], o_sb[:], tc_sb[:], MUL
            )

            # --- interleave production of tile j+1 ---
            if pending_prod is not None:
                production_chunk(pending_prod[0], tl, pending_prod[1], pending_prod[2])
                if tl == 7:
                    pending_prod = None

            # --- interleave heads of tile j-1 ---
            if pending_heads and tl in (1, 3, 5, 6):
                pending_heads.pop(0)()

            if last:
                # final state outputs (before heads() pops hrings[j])
                nc.sync.dma_start(cT_d[:], c_new[:])
                nc.sync.dma_start(hT_d[:], hrings[j][:, c0_ : c0_ + BL])

            if tl == 7:
                assert not pending_heads
                pending_heads = heads(j)
                del psA[j], psB[j], psC[j]

            if not last:
                h_prev, c_prev = h_new, c_new

        # drain remaining heads
        for fn in pending_heads:
            fn()

    nc.compile()
    return nc


def _prep_inputs(x, done, h0, c0, W_ih, W_hh, b_ih, b_hh,
                 W_p1, b_p1, W_p2, b_p2, W_v1, b_v1, W_v2, b_v2):
    """Build per-core input maps (host-side relayout)."""
    x = np.asarray(x, dtype=np.float32)
    done = np.asarray(done, dtype=np.float32)
    h0 = np.asarray(h0, dtype=np.float32)
    c0 = np.asarray(c0, dtype=np.float32)
    W_ih = np.asarray(W_ih, dtype=np.float32)
    W_hh = np.asarray(W_hh, dtype=np.float32)
    b_tot = (np.asarray(b_ih) + np.asarray(b_hh)).astype(np.float32)

    # wih packed: [fc, (g, kc, gu)]
    wih = (
        W_ih.reshape(4, 128, 4, 128).transpose(3, 0, 2, 1).reshape(128, 2048)
    ).copy()
    whh = W_hh.T.copy()  # [128, 512] cols in (i,f,g,o) row order of W_hh

    augw = np.zeros((3, 5 * 128), dtype=np.float32)
    for g in range(4):
        augw[0, g * 128 : (g + 1) * 128] = b_tot[g * 128 : (g + 1) * 128]
    augw[1, GF * 128 : (GF + 1) * 128] = NEG  # f-gate: -1e9 * done_t
    augw[0, 512:640] = b_tot[GO * 128 : (GO + 1) * 128]  # o' bias
    augw[2, 512:640] = NEG  # o': -1e9 * done_{t+1}

    w1 = np.concatenate([np.asarray(W_p1), np.asarray(W_v1)], axis=1).astype(
        np.float32
    )  # [128, 128]
    b1 = np.concatenate([np.asarray(b_p1), np.asarray(b_v1)]).astype(
        np.float32
    ).reshape(128, 1)
    w2 = np.zeros((128, A + 1), dtype=np.float32)
    w2[0:64, 0:A] = np.asarray(W_p2)
    w2[64:128, A] = np.asarray(W_v2)[:, 0]
    b2 = np.concatenate(
        [np.asarray(b_p2), np.asarray(b_v2)]
    ).astype(np.float32).reshape(A + 1, 1)

    done_next = np.concatenate([done[1:], np.zeros((1, B), np.float32)], axis=0)

    in_maps = []
    for k in range(N_CORES):
        sl = slice(k * BL, (k + 1) * BL)
        xk = x[:, sl, :]  # [T, BL, F]
        xT = np.ascontiguousarray(xk.transpose(2, 0, 1).reshape(F, T * BL))
        aug = np.empty((3, T * BL), dtype=np.float32)
        aug[0] = 1.0
        aug[1] = done[:, sl].reshape(-1)
        aug[2] = done_next[:, sl].reshape(-1)
        h0m = (h0[sl] * (1.0 - done[0, sl])[:, None]).T.copy()
        c0k = c0[sl].T.copy()
        in_maps.append(
            dict(
                xT=xT, aug=aug, wih=wih, whh=whh, augw=augw,
                w1=w1, b1=b1, w2=w2, b2=b2, h0m=h0m, c0=c0k,
            )
        )
    return in_maps


_cache = threading.Lock(), {}


def _get_program():
    lock, d = _cache
    with lock:
        if "nc" not in d:
            d["nc"] = build_program(T)
        return d["nc"]


def kernel(**inputs):
    from concourse.bass_utils import run_bass_kernel_spmd

    nc = _get_program()
    in_maps = _prep_inputs(**inputs)
    res = run_bass_kernel_spmd(nc, in_maps, list(range(N_CORES)))

    out = np.empty((T, B, A + 1), dtype=np.float32)
    hT = np.empty((B, H), dtype=np.float32)
    cT = np.empty((B, H), dtype=np.float32)
    for k in range(N_CORES):
        sl = slice(k * BL, (k + 1) * BL)
        r = res.results[k]
        out[:, sl, :] = r["out9"].reshape(A + 1, T, BL).transpose(1, 2, 0)
        hT[sl] = r["hT"].T
        cT[sl] = r["cT"].T
    return out.reshape(T * B, A + 1), hT, cT


# revision 7
# speedup vs baseline: 1.8526x; 1.8526x over previous
"""Trainium2 Bass kernel for the masked-LSTM MemoryAgent problem.

Contract: kernel(**inputs) takes the FULL unsharded inputs (as produced by
setup_inputs()) and returns the full outputs (out [T*B, A+1], hT [B, H],
cT [B, H]) exactly like the reference.

Strategy (8 NeuronCores, data-parallel over the batch/env axis B):
  - Each core owns BL = B/8 = 32 envs and runs the full T=512 sequential scan.
  - Everything on-chip uses a "transposed" layout: partitions = gate/hidden
    units, free dim = (time_local, env).  All elementwise tiles are [128, 32].
  - The input projection pre = x @ W_ih.T (+ biases + done-mask offsets) is
    produced on the tensor engine in batches of 8 timesteps directly into the
    PSUM tiles that the recurrent matmuls later accumulate into, so `pre`
    never round-trips through HBM (memory-optimal: x is read once).
  - The done-mask is folded in algebraically:
      * f-gate:  sigma(gates_f - 1e9*done_t)      == sigma(gates_f) * m_t
      * h-mask:  h_in(t+1) = sigma(gates_o - 1e9*done_{t+1}) * tanh(c)
    so no mask tensors or extra elementwise ops are needed in the scan.
  - The policy/value heads are folded into two packed matmuls (one stationary
    [128,128] for both first layers, one [128,9] block-diagonal for both
    second layers) batched over 8 timesteps.
"""

import os
import sys
import threading

import numpy as np

sys.path.insert(0, "/opt/trn_rl_repo")

import ml_dtypes  # noqa: E402

BF_NP = ml_dtypes.bfloat16

import concourse.bass as bass  # noqa: E402
import concourse.tile as tile  # noqa: E402
from concourse import bacc, mybir  # noqa: E402
from contextlib import ExitStack  # noqa: E402

T, B, F, H, A, MH = 512, 256, 512, 128, 8, 64
G4 = 4 * H  # 512
N_CORES = 8
BL = B // N_CORES  # 32 envs per core
DT = mybir.dt.float32
BF = mybir.dt.bfloat16
NEG = -1.0e9

# gate row ranges in W_ih / W_hh (PyTorch order i, f, g, o)
GI, GF, GG, GO = 0, 1, 2, 3
GATE_ROWS = {GI: (0, 128), GF: (128, 256), GG: (256, 384), GO: (384, 512)}

TSTEPS_PER_TILE = 8
COLS = TSTEPS_PER_TILE * BL  # 256 free columns per psum tile half


def build_program(nt_steps=T):
    """Emit the per-core SPMD program. Returns the compiled bacc module."""
    n_tiles = nt_steps // TSTEPS_PER_TILE
    NTB = nt_steps * BL

    nc = bacc.Bacc(
        "TRN2", target_bir_lowering=False, debug=False, num_devices=N_CORES
    )

    # ---- DRAM I/O ----
    xT_d = nc.dram_tensor("xT", [F, NTB], DT, kind="ExternalInput").ap()
    aug_d = nc.dram_tensor("aug", [3, NTB], DT, kind="ExternalInput").ap()
    wih_d = nc.dram_tensor("wih", [128, 16 * 128], DT, kind="ExternalInput").ap()
    whh_d = nc.dram_tensor("whh", [128, 512], BF, kind="ExternalInput").ap()
    augw_d = nc.dram_tensor("augw", [3, 5 * 128], DT, kind="ExternalInput").ap()
    w1_d = nc.dram_tensor("w1", [128, 128], DT, kind="ExternalInput").ap()
    b1_d = nc.dram_tensor("b1", [128, 1], DT, kind="ExternalInput").ap()
    w2_d = nc.dram_tensor("w2", [128, A + 1], DT, kind="ExternalInput").ap()
    b2_d = nc.dram_tensor("b2", [A + 1, 1], DT, kind="ExternalInput").ap()
    h0_d = nc.dram_tensor("h0m", [128, BL], BF, kind="ExternalInput").ap()
    c0_d = nc.dram_tensor("c0", [128, BL], DT, kind="ExternalInput").ap()

    out9_d = nc.dram_tensor("out9", [A + 1, NTB], DT, kind="ExternalOutput").ap()
    hT_d = nc.dram_tensor("hT", [128, BL], DT, kind="ExternalOutput").ap()
    cT_d = nc.dram_tensor("cT", [128, BL], DT, kind="ExternalOutput").ap()

    SIG = mybir.ActivationFunctionType.Sigmoid
    TANH = mybir.ActivationFunctionType.Tanh
    MUL = mybir.AluOpType.mult
    ADD = mybir.AluOpType.add

    with tile.TileContext(nc) as tc, ExitStack() as ctx:
        consts = ctx.enter_context(tc.tile_pool(name="consts", bufs=1))
        xpool = ctx.enter_context(tc.tile_pool(name="xsb", bufs=8))
        augpool = ctx.enter_context(tc.tile_pool(name="augsb", bufs=2))
        hring_p = ctx.enter_context(tc.tile_pool(name="hring", bufs=2))
        state_p = ctx.enter_context(tc.tile_pool(name="state", bufs=3))
        tmp_p = ctx.enter_context(tc.tile_pool(name="tmp", bufs=3))
        s1_p = ctx.enter_context(tc.tile_pool(name="s1", bufs=2))
        s2_p = ctx.enter_context(tc.tile_pool(name="s2", bufs=2))
        pA_p = ctx.enter_context(tc.tile_pool(name="pA", bufs=2, space="PSUM"))
        pB_p = ctx.enter_context(tc.tile_pool(name="pB", bufs=2, space="PSUM"))
        pC_p = ctx.enter_context(tc.tile_pool(name="pC", bufs=2, space="PSUM"))
        pH1_p = ctx.enter_context(tc.tile_pool(name="pH1", bufs=1, space="PSUM"))
        pH2_p = ctx.enter_context(tc.tile_pool(name="pH2", bufs=1, space="PSUM"))

        # ---- load constants ----
        wih = consts.tile([128, 16 * 128], DT)
        whh = consts.tile([128, 512], BF)
        augw = consts.tile([3, 5 * 128], DT)
        w1 = consts.tile([128, 128], DT)
        b1 = consts.tile([128, 1], DT)
        w2 = consts.tile([128, A + 1], DT)
        b2 = consts.tile([A + 1, 1], DT)
        nc.sync.dma_start(wih[:], wih_d[:])
        nc.sync.dma_start(whh[:], whh_d[:])
        nc.sync.dma_start(augw[:], augw_d[:])
        nc.sync.dma_start(w1[:], w1_d[:])
        nc.sync.dma_start(b1[:], b1_d[:])
        nc.sync.dma_start(w2[:], w2_d[:])
        nc.sync.dma_start(b2[:], b2_d[:])

        h_init = state_p.tile([128, BL], BF, tag="h")
        c_init = state_p.tile([128, BL], DT, tag="c")
        nc.sync.dma_start(h_init[:], h0_d[:])
        nc.sync.dma_start(c_init[:], c0_d[:])

        # psum tiles per production tile j, created lazily
        psA = {}
        psB = {}
        psC = {}
        hrings = {}

        def dma_loads(j):
            """DMA x chunks + aug rows for tile j. Returns tiles."""
            jc = j * COLS
            xs = []
            for kc in range(4):
                t_ = xpool.tile([128, COLS], DT, tag=f"x{kc}")
                nc.sync.dma_start(
                    t_[:], xT_d[kc * 128 : (kc + 1) * 128, jc : jc + COLS]
                )
                xs.append(t_)
            ag = augpool.tile([3, COLS], DT, tag="aug")
            nc.sync.dma_start(ag[:], aug_d[:, jc : jc + COLS])
            return xs, ag

        def production_chunk(j, step_idx, xs, ag):
            """Emit a slice of tile-j pre-production, spread across 8 calls
            (step_idx 0..7)."""
            pa, pb, pc = psA[j], psB[j], psC[j]
            # region list: (psum_ap_256, gate_idx)
            regions = [
                (pa[:, 0:COLS], GI),
                (pa[:, COLS : 2 * COLS], GF),
                (pb[:, 0:COLS], GO),
                (pb[:, COLS : 2 * COLS], GO),  # o' (bias + done_next mask)
                (pc[:, 0:COLS], GG),
            ]
            # 5 regions x (4 x-chunks + 1 aug) = 25 matmuls; spread over
            # step_idx: do region r chunk kc when (r*5 + kc_slot) % 8 == ...
            # simpler: per step_idx emit a fixed sublist.
            all_mms = []
            for r, (ap_, g) in enumerate(regions):
                for kc in range(4):
                    all_mms.append(("x", r, ap_, g, kc))
                all_mms.append(("aug", r, ap_, g, None))
            # split into 8 roughly equal groups, preserving order
            n = len(all_mms)
            lo = (n * step_idx) // 8
            hi = (n * (step_idx + 1)) // 8
            for kind, r, ap_, g, kc in all_mms[lo:hi]:
                if kind == "x":
                    lhs = wih[:, (g * 4 + kc) * 128 : (g * 4 + kc + 1) * 128]
                    # start=True exactly once per psum tile (regions 0/2/4 are
                    # the first region of tiles A/B/C): it marks the whole
                    # 2KiB bank pending-zero, so only the first matmul into
                    # each bank may carry it.
                    nc.tensor.matmul(
                        ap_,
                        lhs,
                        xs[kc][:],
                        start=(kc == 0 and r in (0, 2, 4)),
                        stop=False,
                        skip_group_check=True,
                    )
                else:
                    # aug matmul: region r -> augw col block r
                    awr = (GI, GF, GO, 4, GG)[r]
                    lhs = augw[:, awr * 128 : (awr + 1) * 128]
                    nc.tensor.matmul(
                        ap_,
                        lhs,
                        ag[:],
                        start=False,
                        stop=False,
                        skip_group_check=True,
                    )

        def heads(j):
            """Heads for completed hring tile j; returns list of emit-closures
            to spread across steps."""
            hr = hrings.pop(j)
            jc = j * COLS

            def part1():
                ph1 = pH1_p.tile([128, COLS], DT, tag="ph1")
                nc.tensor.matmul(ph1[:], w1[:], hr[:], start=True, stop=True)
                heads._ph1 = ph1

            def part2():
                s1 = s1_p.tile([128, COLS], DT, tag="s1")
                nc.scalar.activation(
                    s1[:], heads._ph1[:], TANH, bias=b1[:, 0:1]
                )
                heads._s1 = s1

            def part3():
                ph2 = pH2_p.tile([A + 1, COLS], DT, tag="ph2")
                nc.tensor.matmul(
                    ph2[:], w2[:], heads._s1[:], start=True, stop=True
                )
                heads._ph2 = ph2

            def part4():
                s2 = s2_p.tile([A + 1, COLS], DT, tag="s2")
                nc.vector.tensor_scalar_add(s2[:], heads._ph2[:], b2[:, 0:1])
                nc.sync.dma_start(out9_d[:, jc : jc + COLS], s2[:])

            return [part1, part2, part3, part4]

        # ---- prologue: tile 0 loads + production ----
        xs0, ag0 = dma_loads(0)
        psA[0] = pA_p.tile([128, 512], DT, tag="pA", name="psA")
        psB[0] = pB_p.tile([128, 512], DT, tag="pB", name="psB")
        psC[0] = pC_p.tile([128, COLS], DT, tag="pC", name="psC")
        hrings[0] = hring_p.tile([128, COLS], DT, tag="hr", name="hring")
        for s in range(8):
            production_chunk(0, s, xs0, ag0)

        h_prev, c_prev = h_init, c_init
        pending_heads = []
        pending_prod = None  # (j, xs, ag)

        for t in range(nt_steps):
            j, tl = t // TSTEPS_PER_TILE, t % TSTEPS_PER_TILE
            c0_ = tl * BL
            pa, pb, pc = psA[j], psB[j], psC[j]
            last = t == nt_steps - 1

            # --- interleave: start next tile's loads/psum at step 0 ---
            if tl == 0 and j + 1 < n_tiles:
                xs, ag = dma_loads(j + 1)
                psA[j + 1] = pA_p.tile([128, 512], DT, tag="pA", name="psA")
                psB[j + 1] = pB_p.tile([128, 512], DT, tag="pB", name="psB")
                psC[j + 1] = pC_p.tile([128, COLS], DT, tag="pC", name="psC")
                hrings[j + 1] = hring_p.tile([128, COLS], DT, tag="hr", name="hring")
                pending_prod = (j + 1, xs, ag)

            # --- recurrent matmuls (the latency-critical chain) ---
            nc.tensor.matmul(
                pa[:, c0_ : c0_ + BL], whh[:, 0:128], h_prev[:],
                start=False, stop=False, skip_group_check=True,
            )
            nc.tensor.matmul(
                pa[:, COLS + c0_ : COLS + c0_ + BL], whh[:, 128:256], h_prev[:],
                start=False, stop=False, skip_group_check=True,
            )
            nc.tensor.matmul(
                pc[:, c0_ : c0_ + BL], whh[:, 256:384], h_prev[:],
                start=False, stop=False, skip_group_check=True,
            )
            nc.tensor.matmul(
                pb[:, c0_ : c0_ + BL], whh[:, 384:512], h_prev[:],
                start=False, stop=False, skip_group_check=True,
            )
            if not last:
                nc.tensor.matmul(
                    pb[:, COLS + c0_ : COLS + c0_ + BL], whh[:, 384:512], h_prev[:],
                    start=False, stop=False, skip_group_check=True,
                )

            # --- activations ---
            sif = tmp_p.tile([128, 2 * BL], DT, tag="sif")
            pa_if = pa.rearrange("p (g n) -> p g n", g=2)[:, :, c0_ : c0_ + BL]
            nc.scalar.activation(sif[:].rearrange("p (g n) -> p g n", g=2), pa_if, SIG)
            g_sb = tmp_p.tile([128, BL], DT, tag="g")
            nc.scalar.activation(g_sb[:], pc[:, c0_ : c0_ + BL], TANH)
            oop = tmp_p.tile([128, 2 * BL], DT, tag="oop")
            if not last:
                pb_oo = pb.rearrange("p (g n) -> p g n", g=2)[:, :, c0_ : c0_ + BL]
                nc.scalar.activation(
                    oop[:].rearrange("p (g n) -> p g n", g=2), pb_oo, SIG
                )
            else:
                nc.scalar.activation(oop[:, 0:BL], pb[:, c0_ : c0_ + BL], SIG)

            # --- cell update (DVE) ---
            t1 = tmp_p.tile([128, BL], DT, tag="t1")
            nc.vector.tensor_tensor(t1[:], sif[:, BL : 2 * BL], c_prev[:], MUL)
            t2 = tmp_p.tile([128, BL], DT, tag="t2")
            nc.vector.tensor_tensor(t2[:], sif[:, 0:BL], g_sb[:], MUL)
            c_new = state_p.tile([128, BL], DT, tag="c")
            nc.vector.tensor_tensor(c_new[:], t1[:], t2[:], ADD)
            tc_sb = tmp_p.tile([128, BL], DT, tag="tc")
            nc.scalar.activation(tc_sb[:], c_new[:], TANH)
            if not last:
                h_new = state_p.tile([128, BL], BF, tag="h")
                nc.vector.tensor_tensor(h_new[:], oop[:, BL : 2 * BL], tc_sb[:], MUL)
            # unmasked h for outputs/heads
            nc.vector.tensor_tensor(
                hrings[j][:, c0_ : c0_ + BL], oop[:, 0:BL], tc_sb[:], MUL
            )

            # --- interleave production of tile j+1 ---
            if pending_prod is not None:
                production_chunk(pending_prod[0], tl, pending_prod[1], pending_prod[2])
                if tl == 7:
                    pending_prod = None

            # --- interleave heads of tile j-1 ---
            if pending_heads and tl in (1, 3, 5, 6):
                pending_heads.pop(0)()

            if last:
                # final state outputs (before heads() pops hrings[j])
                nc.sync.dma_start(cT_d[:], c_new[:])
                nc.sync.dma_start(hT_d[:], hrings[j][:, c0_ : c0_ + BL])

            if tl == 7:
                assert not pending_heads
                pending_heads = heads(j)
                del psA[j], psB[j], psC[j]

            if not last:
                h_prev, c_prev = h_new, c_new

        # drain remaining heads
        for fn in pending_heads:
            fn()

    nc.compile()
    return nc


def _prep_inputs(x, done, h0, c0, W_ih, W_hh, b_ih, b_hh,
                 W_p1, b_p1, W_p2, b_p2, W_v1, b_v1, W_v2, b_v2):
    """Build per-core input maps (host-side relayout)."""
    x = np.asarray(x, dtype=np.float32)
    done = np.asarray(done, dtype=np.float32)
    h0 = np.asarray(h0, dtype=np.float32)
    c0 = np.asarray(c0, dtype=np.float32)
    W_ih = np.asarray(W_ih, dtype=np.float32)
    W_hh = np.asarray(W_hh, dtype=np.float32)
    b_tot = (np.asarray(b_ih) + np.asarray(b_hh)).astype(np.float32)

    # wih packed: [fc, (g, kc, gu)]
    wih = (
        W_ih.reshape(4, 128, 4, 128).transpose(3, 0, 2, 1).reshape(128, 2048)
    ).astype(np.float32)
    whh = W_hh.T.astype(BF_NP)  # [128, 512] cols in (i,f,g,o) row order

    augw = np.zeros((3, 5 * 128), dtype=np.float32)
    for g in range(4):
        augw[0, g * 128 : (g + 1) * 128] = b_tot[g * 128 : (g + 1) * 128]
    augw[1, GF * 128 : (GF + 1) * 128] = NEG  # f-gate: -1e9 * done_t
    augw[0, 512:640] = b_tot[GO * 128 : (GO + 1) * 128]  # o' bias
    augw[2, 512:640] = NEG  # o': -1e9 * done_{t+1}

    w1 = np.concatenate([np.asarray(W_p1), np.asarray(W_v1)], axis=1).astype(
        np.float32
    )  # [128, 128]
    b1 = np.concatenate([np.asarray(b_p1), np.asarray(b_v1)]).astype(
        np.float32
    ).reshape(128, 1)
    w2 = np.zeros((128, A + 1), dtype=np.float32)
    w2[0:64, 0:A] = np.asarray(W_p2)
    w2[64:128, A] = np.asarray(W_v2)[:, 0]
    b2 = np.concatenate(
        [np.asarray(b_p2), np.asarray(b_v2)]
    ).astype(np.float32).reshape(A + 1, 1)

    done_next = np.concatenate([done[1:], np.zeros((1, B), np.float32)], axis=0)

    in_maps = []
    for k in range(N_CORES):
        sl = slice(k * BL, (k + 1) * BL)
        xk = x[:, sl, :]  # [T, BL, F]
        xT = np.ascontiguousarray(xk.transpose(2, 0, 1).reshape(F, T * BL))
        aug = np.empty((3, T * BL), dtype=np.float32)
        aug[0] = 1.0
        aug[1] = done[:, sl].reshape(-1)
        aug[2] = done_next[:, sl].reshape(-1)
        h0m = ((h0[sl] * (1.0 - done[0, sl])[:, None]).T).astype(BF_NP)
        c0k = c0[sl].T.copy()
        in_maps.append(
            dict(
                xT=xT, aug=aug, wih=wih, whh=whh, augw=augw,
                w1=w1, b1=b1, w2=w2, b2=b2, h0m=h0m, c0=c0k,
            )
        )
    return in_maps


_cache = threading.Lock(), {}


def _get_program():
    lock, d = _cache
    with lock:
        if "nc" not in d:
            d["nc"] = build_program(T)
        return d["nc"]


def kernel(**inputs):
    from concourse.bass_utils import run_bass_kernel_spmd

    nc = _get_program()
    in_maps = _prep_inputs(**inputs)
    res = run_bass_kernel_spmd(nc, in_maps, list(range(N_CORES)))

    out = np.empty((T, B, A + 1), dtype=np.float32)
    hT = np.empty((B, H), dtype=np.float32)
    cT = np.empty((B, H), dtype=np.float32)
    for k in range(N_CORES):
        sl = slice(k * BL, (k + 1) * BL)
        r = res.results[k]
        out[:, sl, :] = r["out9"].reshape(A + 1, T, BL).transpose(1, 2, 0)
        hT[sl] = r["hT"].T
        cT[sl] = r["cT"].T
    return out.reshape(T * B, A + 1), hT, cT
